# revision 15
# baseline (speedup 1.0000x reference)
"""Chamfer loss (K=8 KNN mean-distance, both directions) on 8 Trainium2 cores.

Strategy
--------
8 independent work units = (batch b in 0..3) x (direction d in 0..1), one per
NeuronCore.  A unit computes, for each of its 8192 query points, the 8 smallest
squared euclidean distances to its 8192 target points, entirely on-chip:

  * The TensorEngine computes s[n, m] = 2*q_n . p_m - |p_m|^2 as a single
    K=12 bf16 matmul per [128 x 512] tile (hi/lo bf16 splitting keeps absolute
    error ~1e-5, far below what the final reduction can see).  The per-row
    constant |q_n|^2 is left out: it does not change each row's top-8
    selection, and the host adds it back afterwards.
  * The VectorEngine's hardware top-8 instruction (InstMax) reads the PSUM
    tiles directly and produces each row's 8 largest s values (= 8 smallest
    d2).  A second tiny InstMax merges the 4 per-group candidates.
  * Host side: d2 = q2 - s, dist = sqrt(max(d2, 0)), then the scalar mean.

No collectives: each core returns a [128, 512] tile of top-8 values and the
host reduces 8 scalars.
"""

import numpy as np

B = 4
N = 8192
K = 8
NCORES = 8
KDIM = 12      # matmul contraction rows after bf16 hi/lo splitting
PT = 128       # partition tile (queries per row-tile)
NT = N // PT   # 64 row-tiles
MMF = 512      # matmul moving free dim (one PSUM bank of f32)
GW = 2048      # InstMax group width (4 PSUM banks)
PSUM_BUFS = 2  # psum pool double buffering
NG = N // GW   # 4 groups per row-tile

_CACHE = {}


def _split_multiwaits(nc, mybir):
    """Split waits that span >1 semaphore onto a preceding same-engine NoOp.

    Engine-queue ISA structs (Matmult, Max/BN, ...) hold a single sync-wait
    slot; walrus rejects instructions carrying waits on two semaphores.  The
    engine sequencer dispatches in order, so hoisting the extra waits onto a
    NoOp immediately before the instruction is semantically identical.
    """
    nid = 0
    for blk in nc.main_func.blocks:
        il = blk.instructions
        new = []
        for ins in il:
            si = ins.sync_info
            waits = list(si.on_wait) if (si is not None and si.on_wait) else []
            if len(waits) > 1:
                engname = str(ins.engine).split(".")[-1]
                keep = next(
                    (w for w in waits if (w.ant_name or "").startswith(engname)),
                    waits[-1],
                )
                for w in waits:
                    if w is keep:
                        continue
                    nop = mybir.InstNoOp(name=f"I-waitsplit-{nid}", ins=[], outs=[])
                    nid += 1
                    nop.engine = ins.engine
                    nop.sync_info = mybir.SyncInfo(on_wait=[w], on_update=[])
                    new.append(nop)
                ins.sync_info = mybir.SyncInfo(
                    on_wait=[keep],
                    on_update=list(si.on_update) if si.on_update else [],
                )
            new.append(ins)
        il[:] = new


def _build_nc():
    import concourse.bass as bass
    import concourse.mybir as mybir
    import concourse.tile as tile

    nc = bass.Bass()
    qt = nc.dram_tensor("qt", [KDIM, N], mybir.dt.bfloat16, kind="ExternalInput")
    pt = nc.dram_tensor("pt", [KDIM, N], mybir.dt.bfloat16, kind="ExternalInput")
    # Per row-tile and PSUM group, the top-8 candidates (NG*K per row); the
    # final 32 -> 8 merge happens on the host, off the critical path.
    out = nc.dram_tensor(
        "out", [PT, NT * NG * K], mybir.dt.float32, kind="ExternalOutput"
    )

    with tile.TileContext(nc) as tc:
        with (
            tc.tile_pool(name="singles", bufs=1) as singles,
            tc.tile_pool(name="psum", bufs=1, space=bass.MemorySpace.PSUM) as psum_pool,
        ):
            qts = singles.tile([KDIM, N], mybir.dt.bfloat16)
            pts = singles.tile([KDIM, N], mybir.dt.bfloat16)
            stage = singles.tile([PT, NT * NG * K], mybir.dt.float32)
            nc.sync.dma_start(out=qts[:], in_=qt[:])
            nc.sync.dma_start(out=pts[:], in_=pt[:])
            # Two persistent, distinctly-tagged PSUM tiles alternated manually:
            # the pool's own slot allocator may hand the *same* slot to
            # consecutive groups (bufs only caps concurrency), which would
            # serialize PE behind DVE on every group.
            ps_tiles = [
                psum_pool.tile([PT, GW], mybir.dt.float32, tag=f"ps{i}", name=f"ps{i}")
                for i in range(PSUM_BUFS)
            ]

            gi = 0
            for t in range(NT):
                for g in range(NG):
                    ps = ps_tiles[gi % PSUM_BUFS]
                    for j in range(GW // MMF):
                        m0 = g * GW + j * MMF
                        nc.tensor.matmul(
                            ps[:, j * MMF : (j + 1) * MMF],
                            qts[:, t * PT : (t + 1) * PT],
                            pts[:, m0 : m0 + MMF],
                            start=True,
                            stop=True,
                        )
                    nc.vector.max(out=stage[:, gi * K : (gi + 1) * K], in_=ps)
                    gi += 1
            nc.sync.dma_start(out=out[:], in_=stage[:])

    import concourse.mybir as mybir_mod

    _split_multiwaits(nc, mybir_mod)
    return nc


def _get_nc():
    if "nc" not in _CACHE:
        _CACHE["nc"] = _build_nc()
    return _CACHE["nc"]


def _bf16_split(x64, levels):
    """Split float64 array into `levels` bf16 arrays summing to ~x64."""
    import ml_dtypes

    parts = []
    r = x64
    for _ in range(levels):
        h = r.astype(ml_dtypes.bfloat16)
        parts.append(h)
        r = r - h.astype(np.float64)
    return parts


def _core_inputs(q32, p32):
    """Build the [KDIM, N] bf16 lhsT/rhs feature blocks for one unit.

    s[n, m] = sum_k QT[k, n] * PT[k, m] = 2*q_n.p_m - |p_m|^2
    """
    import ml_dtypes

    q64 = q32.astype(np.float64)
    p64 = p32.astype(np.float64)
    qh, ql = _bf16_split(q64, 2)  # [N, 3] each
    ph, pl = _bf16_split(p64, 2)
    p2 = (p64 * p64).sum(-1)  # [N]
    p2h, p2m, p2l = _bf16_split(p2, 3)

    bf = ml_dtypes.bfloat16
    ones = np.ones(N, dtype=bf)
    QT = np.empty((KDIM, N), dtype=bf)
    PTm = np.empty((KDIM, N), dtype=bf)
    for d in range(3):
        QT[d] = qh[:, d]
        QT[3 + d] = qh[:, d]
        QT[6 + d] = ql[:, d]
        # x2 scaling is exact in bf16
        PTm[d] = (2.0 * ph[:, d].astype(np.float32)).astype(bf)
        PTm[3 + d] = (2.0 * pl[:, d].astype(np.float32)).astype(bf)
        PTm[6 + d] = PTm[d]
    QT[9] = ones
    QT[10] = ones
    QT[11] = ones
    PTm[9] = (-p2h.astype(np.float32)).astype(bf)
    PTm[10] = (-p2m.astype(np.float32)).astype(bf)
    PTm[11] = (-p2l.astype(np.float32)).astype(bf)
    return QT, PTm


def _run(pc_source, pc_target, pred_flow, trace=False):
    from concourse.bass_utils import run_bass_kernel_spmd

    pc_source = np.asarray(pc_source, dtype=np.float32)
    pc_target = np.asarray(pc_target, dtype=np.float32)
    pred_flow = np.asarray(pred_flow, dtype=np.float32)
    pc_pred = pc_source + pred_flow  # f32, matching the reference

    in_maps = []
    q2s = []
    for c in range(NCORES):
        b, d = divmod(c, 2)
        if d == 0:
            q32, p32 = pc_pred[b], pc_target[b]
        else:
            q32, p32 = pc_target[b], pc_pred[b]
        QT, PTm = _core_inputs(q32, p32)
        in_maps.append({"qt": QT, "pt": PTm})
        q2s.append((q32.astype(np.float64) ** 2).sum(-1))  # [N]

    nc = _get_nc()
    res = run_bass_kernel_spmd(nc, in_maps, list(range(NCORES)), trace=trace)

    total = 0.0
    for c in range(NCORES):
        v = np.asarray(res.results[c]["out"], dtype=np.float64)  # [128, NT*NG*K]
        # v[p, (t*NG+g)*K + k]: k-th largest s of group g for query t*128 + p
        v = v.reshape(PT, NT, NG * K).transpose(1, 0, 2).reshape(N, NG * K)
        v = np.partition(v, NG * K - K - 1, axis=1)[:, -K:]  # 8 largest of 32
        d2 = q2s[c][:, None] - v
        np.maximum(d2, 0.0, out=d2)
        total += np.sqrt(d2).sum()

    loss = total / float(B * N * K)
    return np.asarray(loss, dtype=np.float32), res


def kernel(pc_source, pc_target, pred_flow):
    loss, _ = _run(pc_source, pc_target, pred_flow, trace=False)
    return loss


# revision 18
# speedup vs baseline: 1.0397x; 1.0397x over previous
"""Chamfer loss (K=8 KNN mean-distance, both directions) on 8 Trainium2 cores.

Strategy
--------
8 independent work units = (batch b in 0..3) x (direction d in 0..1), one per
NeuronCore.  A unit computes, for each of its 8192 query points, the 8 smallest
squared euclidean distances to its 8192 target points, entirely on-chip:

  * The TensorEngine computes s[n, m] = 2*q_n . p_m - |p_m|^2 as a single
    K=12 bf16 matmul per [128 x 512] tile (hi/lo bf16 splitting keeps absolute
    error ~1e-5, far below what the final reduction can see).  The per-row
    constant |q_n|^2 is left out: it does not change each row's top-8
    selection, and the host adds it back afterwards.
  * The ScalarEngine evacuates each 4-bank PSUM group into an SBUF strip
    (absorbing the PSUM access latency on the otherwise idle ACT engine).
  * The VectorEngine's hardware top-8 instruction (InstMax) then runs one
    [128 x 8192] scan per row-tile, yielding each row's 8 largest s values
    (= 8 smallest d2).  DVE at 1 elem/lane/cycle is the structural floor.
  * Host side: d2 = q2 - s, dist = sqrt(max(d2, 0)), then the scalar mean.

No collectives: each core returns a [128, 512] tile of top-8 values and the
host reduces 8 scalars.
"""

import numpy as np

B = 4
N = 8192
K = 8
NCORES = 8
KDIM = 12      # matmul contraction rows after bf16 hi/lo splitting
PT = 128       # partition tile (queries per row-tile)
NT = N // PT   # 64 row-tiles
MMF = 512      # matmul moving free dim (one PSUM bank of f32)
GW = 2048      # InstMax group width (4 PSUM banks)
PSUM_BUFS = 2  # psum pool double buffering
NG = N // GW   # 4 groups per row-tile

_CACHE = {}


def _split_multiwaits(nc, mybir):
    """Split waits that span >1 semaphore onto a preceding same-engine NoOp.

    Engine-queue ISA structs (Matmult, Max/BN, ...) hold a single sync-wait
    slot; walrus rejects instructions carrying waits on two semaphores.  The
    engine sequencer dispatches in order, so hoisting the extra waits onto a
    NoOp immediately before the instruction is semantically identical.
    """
    nid = 0
    for blk in nc.main_func.blocks:
        il = blk.instructions
        new = []
        for ins in il:
            si = ins.sync_info
            waits = list(si.on_wait) if (si is not None and si.on_wait) else []
            if len(waits) > 1:
                engname = str(ins.engine).split(".")[-1]
                keep = next(
                    (w for w in waits if (w.ant_name or "").startswith(engname)),
                    waits[-1],
                )
                for w in waits:
                    if w is keep:
                        continue
                    nop = mybir.InstNoOp(name=f"I-waitsplit-{nid}", ins=[], outs=[])
                    nid += 1
                    nop.engine = ins.engine
                    nop.sync_info = mybir.SyncInfo(on_wait=[w], on_update=[])
                    new.append(nop)
                ins.sync_info = mybir.SyncInfo(
                    on_wait=[keep],
                    on_update=list(si.on_update) if si.on_update else [],
                )
            new.append(ins)
        il[:] = new


def _build_nc():
    import concourse.bass as bass
    import concourse.mybir as mybir
    import concourse.tile as tile

    nc = bass.Bass()
    qt = nc.dram_tensor("qt", [KDIM, N], mybir.dt.bfloat16, kind="ExternalInput")
    pt = nc.dram_tensor("pt", [KDIM, N], mybir.dt.bfloat16, kind="ExternalInput")
    out = nc.dram_tensor("out", [PT, NT * K], mybir.dt.float32, kind="ExternalOutput")

    with tile.TileContext(nc) as tc:
        with (
            tc.tile_pool(name="singles", bufs=1) as singles,
            tc.tile_pool(name="psum", bufs=1, space=bass.MemorySpace.PSUM) as psum_pool,
        ):
            qts = singles.tile([KDIM, N], mybir.dt.bfloat16)
            pts = singles.tile([KDIM, N], mybir.dt.bfloat16)
            stage = singles.tile([PT, NT * K], mybir.dt.float32)
            nc.sync.dma_start(out=qts[:], in_=qt[:])
            nc.sync.dma_start(out=pts[:], in_=pt[:])
            # Persistent, distinctly-tagged ping-pong tiles (the pool's own
            # slot allocator may hand the *same* slot to consecutive groups,
            # serializing the pipeline).
            ps_tiles = [
                psum_pool.tile([PT, GW], mybir.dt.float32, tag=f"ps{i}", name=f"ps{i}")
                for i in range(PSUM_BUFS)
            ]
            sb_tiles = [
                singles.tile([PT, N], mybir.dt.float32, tag=f"sb{i}", name=f"sb{i}")
                for i in range(2)
            ]

            gi = 0
            for t in range(NT):
                sb = sb_tiles[t % 2]
                for g in range(NG):
                    ps = ps_tiles[gi % PSUM_BUFS]
                    for j in range(GW // MMF):
                        m0 = g * GW + j * MMF
                        nc.tensor.matmul(
                            ps[:, j * MMF : (j + 1) * MMF],
                            qts[:, t * PT : (t + 1) * PT],
                            pts[:, m0 : m0 + MMF],
                            start=True,
                            stop=True,
                        )
                    # ScalarE evacuates the PSUM group into the row-tile's
                    # SBUF strip; the (otherwise idle) ACT engine absorbs the
                    # PSUM access latency and frees DVE to run one big top-8
                    # scan per row-tile.
                    nc.scalar.copy(out=sb[:, g * GW : (g + 1) * GW], in_=ps[:])
                    gi += 1
                nc.vector.max(out=stage[:, t * K : (t + 1) * K], in_=sb)
            nc.sync.dma_start(out=out[:], in_=stage[:])

    import concourse.mybir as mybir_mod

    _split_multiwaits(nc, mybir_mod)
    return nc


def _get_nc():
    if "nc" not in _CACHE:
        _CACHE["nc"] = _build_nc()
    return _CACHE["nc"]


def _bf16_split(x64, levels):
    """Split float64 array into `levels` bf16 arrays summing to ~x64."""
    import ml_dtypes

    parts = []
    r = x64
    for _ in range(levels):
        h = r.astype(ml_dtypes.bfloat16)
        parts.append(h)
        r = r - h.astype(np.float64)
    return parts


def _core_inputs(q32, p32):
    """Build the [KDIM, N] bf16 lhsT/rhs feature blocks for one unit.

    s[n, m] = sum_k QT[k, n] * PT[k, m] = 2*q_n.p_m - |p_m|^2
    """
    import ml_dtypes

    q64 = q32.astype(np.float64)
    p64 = p32.astype(np.float64)
    qh, ql = _bf16_split(q64, 2)  # [N, 3] each
    ph, pl = _bf16_split(p64, 2)
    p2 = (p64 * p64).sum(-1)  # [N]
    p2h, p2m, p2l = _bf16_split(p2, 3)

    bf = ml_dtypes.bfloat16
    ones = np.ones(N, dtype=bf)
    QT = np.empty((KDIM, N), dtype=bf)
    PTm = np.empty((KDIM, N), dtype=bf)
    for d in range(3):
        QT[d] = qh[:, d]
        QT[3 + d] = qh[:, d]
        QT[6 + d] = ql[:, d]
        # x2 scaling is exact in bf16
        PTm[d] = (2.0 * ph[:, d].astype(np.float32)).astype(bf)
        PTm[3 + d] = (2.0 * pl[:, d].astype(np.float32)).astype(bf)
        PTm[6 + d] = PTm[d]
    QT[9] = ones
    QT[10] = ones
    QT[11] = ones
    PTm[9] = (-p2h.astype(np.float32)).astype(bf)
    PTm[10] = (-p2m.astype(np.float32)).astype(bf)
    PTm[11] = (-p2l.astype(np.float32)).astype(bf)
    return QT, PTm


def _run(pc_source, pc_target, pred_flow, trace=False):
    from concourse.bass_utils import run_bass_kernel_spmd

    pc_source = np.asarray(pc_source, dtype=np.float32)
    pc_target = np.asarray(pc_target, dtype=np.float32)
    pred_flow = np.asarray(pred_flow, dtype=np.float32)
    pc_pred = pc_source + pred_flow  # f32, matching the reference

    in_maps = []
    q2s = []
    for c in range(NCORES):
        b, d = divmod(c, 2)
        if d == 0:
            q32, p32 = pc_pred[b], pc_target[b]
        else:
            q32, p32 = pc_target[b], pc_pred[b]
        QT, PTm = _core_inputs(q32, p32)
        in_maps.append({"qt": QT, "pt": PTm})
        q2s.append((q32.astype(np.float64) ** 2).sum(-1))  # [N]

    nc = _get_nc()
    try:
        res = run_bass_kernel_spmd(nc, in_maps, list(range(NCORES)), trace=trace)
    except Exception:
        # One retry for transient device errors (e.g. a wedged core left over
        # from a previous session); re-raises if it persists.
        import time as _time

        _time.sleep(3.0)
        res = run_bass_kernel_spmd(nc, in_maps, list(range(NCORES)), trace=trace)

    total = 0.0
    for c in range(NCORES):
        v = np.asarray(res.results[c]["out"], dtype=np.float64)  # [128, NT*K]
        # v[p, t*K + k] is the k-th largest s for query n = t*128 + p
        v = v.reshape(PT, NT, K).transpose(1, 0, 2).reshape(N, K)
        d2 = q2s[c][:, None] - v
        np.maximum(d2, 0.0, out=d2)
        total += np.sqrt(d2).sum()

    loss = total / float(B * N * K)
    return np.asarray(loss, dtype=np.float32), res


def kernel(pc_source, pc_target, pred_flow):
    loss, _ = _run(pc_source, pc_target, pred_flow, trace=False)
    return loss
